# revision 1
# baseline (speedup 1.0000x reference)
"""AttentionBlock (GroupNorm -> QKV 1x1 conv -> softmax attention -> proj conv
-> residual) as a Bass/Tile kernel for 8 Trainium2 NeuronCores.

Sharding: core c handles batch b=c//2, query-half hf=c%2 (2048 of 4096 tokens).
Host permutes each core's x so its query half is always columns 0:2048 (keys are
permutation-invariant under softmax attention), making the program identical on
every core (SPMD). K and V are computed for the full 4096 tokens on both cores
of a batch (duplicated conv work, no collectives needed).

GroupNorm is folded into the conv weights: H = A*x + B per channel, so
  conv(H) = (W diag(A)) @ x + (W @ B + b)
The weight row-scaling and effective biases are computed on-device after the
stats pass; the conv matmuls then consume raw x tiles directly (no DVE in the
conv critical path).

Math layout notes (PE matmul computes out[M,N] = lhsT[K,M].T @ rhs[K,N],
contraction K on partitions):
  - K conv:  lhsT=WkT_eff[ci,co], rhs=x[ci]      -> K  [c, n]  (resident SBUF)
  - V conv:  lhsT=x[ci] n-slice, rhs=WvT_eff[ci] -> Vt [n, c]  (resident SBUF)
  - Q conv:  like K, first 2048 cols only        -> Q  [c, nq] (DRAM scratch)
  - S^T = K^T Q: lhsT=K[:,nk], rhs=Q[:,qb]       -> St [nk, nq]  (psum)
  - P^T = exp(St/sqrt(C))  (no max subtraction: scores ~ N(0,1), safe in f32)
  - rowsum = ones.T @ P^T  (psum row [1, nq], accumulated over nk)
  - PV: lhsT=Vt[nk] c-slice, rhs=P^T             -> attn [c, nq] (psum, acc nk)
  - proj on unnormalized attn; 1/rowsum applied to the proj OUTPUT columns via
    a partition-broadcast reciprocal tile (softmax scale commutes with proj).
All big matmuls run as float32r (1 cycle/row at N=512; tf32-like precision).
DRAM scratch round-trips (qsc/vbias/rsc) carry explicit sync deps: Tile does
not track raw-DRAM RAW hazards and the HWDGE queues complete out of order.
"""

import functools
import sys
from contextlib import ExitStack

import numpy as np


def _imports():
    try:
        import concourse.bass  # noqa: F401
    except ImportError:
        sys.path.insert(0, "/opt/trn_rl_repo")
    import concourse.bass as bass
    import concourse.tile as tile
    from concourse import bacc, mybir
    from concourse.bass_utils import run_bass_kernel_spmd
    from concourse.masks import make_identity

    return bass, bacc, tile, mybir, run_bass_kernel_spmd, make_identity


P = 128          # partitions
C = 512          # channels
CT = C // P      # 4 channel tiles
N = 4096         # tokens per batch (64*64)
NQ = 2048        # queries per core
NB = 512         # n-chunk width (one psum bank of f32)
NCH = N // NB    # 8 n-chunks
QBW = 512        # query block width
NQB = NQ // QBW  # 4 query blocks
NKT = N // P     # 32 key tiles
G = 32           # groups
GSZ = C // G     # 16 channels per group
EPS = 1e-5
ISQ = 1.0 / float(np.sqrt(C))


def _build_body(nc, tc, ctx, bass, tile, mybir):
    from concourse.tile import add_dep_helper

    f32 = mybir.dt.float32
    f32r = mybir.dt.float32r
    AF = mybir.ActivationFunctionType

    x_d = nc._io["x"]
    w_d = nc._io["w"]
    b_d = nc._io["b"]
    nw_d = nc._io["nw"]
    nb_d = nc._io["nb"]
    gm_d = nc._io["gm"]
    gmT_d = nc._io["gmT"]
    out_d = nc._io["out"]
    qsc_d = nc._io["qsc"]
    vbias_d = nc._io["vbias"]
    rsc_d = nc._io["rsc"]
    pools = nc._pools
    consts = pools["consts"]
    kvp = pools["kv"]
    xio = pools["xio"]
    qs = pools["qs"]
    ptp = pools["ptp"]
    fop = pools["fop"]
    stats = pools["stats"]
    bstp = pools["bstp"]
    ps_work = pools["ps_work"]
    ps_out = pools["ps_out"]

    # ---- constants ----
    wsb = {}  # wsb[m][ci] = [128 c_in, 512 c_out]; wp shares wq slots (tag)
    for m in "qkv":
        wsb[m] = []
        for ci in range(CT):
            t = consts.tile([P, C], f32r, tag=f"w{m}{ci}", name=f"w{m}{ci}")
            nc.gpsimd.dma_start(out=t, in_=w_d[m][ci * P : (ci + 1) * P, :])
            wsb[m].append(t)
    bsb = {}  # per-partition bias columns [128,1] per c_out tile
    for m in "qkvp":
        bsb[m] = []
        for co in range(CT):
            t = consts.tile([P, 1], f32, tag=f"b{m}{co}", name=f"b{m}{co}")
            nc.gpsimd.dma_start(out=t, in_=b_d[m][co * P : (co + 1) * P, :])
            bsb[m].append(t)
    nwsb, nbsb = [], []
    for ci in range(CT):
        t = consts.tile([P, 1], f32, tag=f"nw{ci}", name=f"nw{ci}")
        nc.gpsimd.dma_start(out=t, in_=nw_d[ci * P : (ci + 1) * P, :])
        nwsb.append(t)
        t = consts.tile([P, 1], f32, tag=f"nb{ci}", name=f"nb{ci}")
        nc.gpsimd.dma_start(out=t, in_=nb_d[ci * P : (ci + 1) * P, :])
        nbsb.append(t)
    gmsb = []
    gmTsb = []
    for ci in range(CT):
        t = consts.tile([P, G], f32, tag=f"gm{ci}", name=f"gm{ci}")
        nc.gpsimd.dma_start(out=t, in_=gm_d[ci, :, :])
        gmsb.append(t)
        t2 = consts.tile([G, P], f32, tag=f"gmT{ci}", name=f"gmT{ci}")
        nc.gpsimd.dma_start(out=t2, in_=gmT_d[ci, :, :])
        gmTsb.append(t2)
    ones_f32 = consts.tile([P, 1], f32, tag="ones_f32", name="ones_f32")
    nc.vector.memset(ones_f32, 1.0)
    ones_col = consts.tile([P, 1], f32r, tag="ones_col", name="ones_col")
    nc.vector.tensor_copy(ones_col, ones_f32)
    eps32 = consts.tile([G, 1], f32, tag="eps32", name="eps32")
    nc.vector.memset(eps32, EPS)

    # ---- stage 1: GroupNorm statistics (streaming pass over x) ----
    mv = []
    for ci in range(CT):
        bst = bstp.tile([P, NCH, 6], f32, tag="bst", name=f"bst{ci}")
        for j in range(NCH):
            xt = xio.tile([P, NB], f32r, tag="x", name=f"x1_{ci}_{j}")
            eng = nc.sync if (ci * NCH + j) % 2 == 0 else nc.scalar
            eng.dma_start(
                out=xt, in_=x_d[ci * P : (ci + 1) * P, j * NB : (j + 1) * NB]
            )
            nc.vector.bn_stats(out=bst[:, j, :], in_=xt)
        m = stats.tile([P, 2], f32, tag=f"mv{ci}", name=f"mv{ci}")
        nc.vector.bn_aggr(out=m, in_=bst)
        # m[:,1] := var + mean^2 = E[x^2]
        tmp = stats.tile([P, 1], f32, tag=f"tmp{ci}", name=f"tmp{ci}")
        nc.vector.tensor_mul(tmp, m[:, 0:1], m[:, 0:1])
        nc.vector.tensor_add(m[:, 1:2], m[:, 1:2], tmp)
        mv.append(m)
    ps_g = ps_work.tile([G, 2], f32, tag="work", name="psg")
    for ci in range(CT):
        nc.tensor.matmul(
            ps_g, lhsT=gmsb[ci], rhs=mv[ci], start=(ci == 0), stop=(ci == CT - 1)
        )
    gs = stats.tile([G, 2], f32, tag="gs", name="gs")  # [gmean, gE[x^2]]
    nc.vector.tensor_copy(gs, ps_g)
    gvar = stats.tile([G, 1], f32, tag="gvar", name="gvar")
    nc.vector.tensor_mul(gvar, gs[:, 0:1], gs[:, 0:1])
    nc.vector.tensor_sub(gvar, gs[:, 1:2], gvar)
    grstd = stats.tile([G, 1], f32, tag="grstd", name="grstd")
    nc.scalar.activation(out=grstd, in_=gvar, func=AF.Sqrt, bias=eps32, scale=1.0)
    nc.vector.reciprocal(grstd, grstd)
    # broadcast group stats to channels via binary-mask matmul
    gsr = stats.tile([G, 2], f32, tag="gsr", name="gsr")  # [gmean, grstd]
    nc.vector.tensor_copy(gsr[:, 0:1], gs[:, 0:1])
    nc.vector.tensor_copy(gsr[:, 1:2], grstd)
    Asb, Bsb = [], []
    for ci in range(CT):
        mrps = ps_work.tile([P, 2], f32, tag="work", name=f"mrps{ci}")
        nc.tensor.matmul(mrps, lhsT=gmTsb[ci], rhs=gsr, start=True, stop=True)
        mr = stats.tile([P, 2], f32, tag=f"mr{ci}", name=f"mr{ci}")
        nc.vector.tensor_copy(mr, mrps)
        a = stats.tile([P, 1], f32, tag=f"A{ci}", name=f"A{ci}")
        nc.vector.tensor_mul(a, mr[:, 1:2], nwsb[ci])
        bb = stats.tile([P, 1], f32, tag=f"Bf{ci}", name=f"Bf{ci}")
        nc.vector.tensor_mul(bb, mr[:, 0:1], a)
        nc.vector.tensor_sub(bb, nbsb[ci], bb)
        # fp32r matmul moving operand needs an even free count: duplicate
        b2 = stats.tile([P, 2], f32r, tag=f"B{ci}", name=f"B{ci}")
        nc.vector.tensor_copy(b2[:, 0:1], bb)
        nc.vector.tensor_copy(b2[:, 1:2], bb)
        Asb.append(a)
        Bsb.append(b2)

    # ---- effective biases: beff[m] = W_m @ B + b_m  (uses UNSCALED W) ----
    beff = {}
    for m in "qkv":
        beff[m] = []
        for co in range(CT):
            bp = ps_work.tile([P, 2], f32, tag="work", name=f"beffps{m}{co}")
            for ci in range(CT):
                nc.tensor.matmul(
                    bp, lhsT=wsb[m][ci][:, co * P : (co + 1) * P], rhs=Bsb[ci],
                    start=(ci == 0), stop=(ci == CT - 1),
                )
            bt = stats.tile([P, 1], f32, tag=f"beff{m}{co}", name=f"beff{m}{co}")
            nc.vector.tensor_add(bt, bp[:, 0:1], bsb[m][co])
            beff[m].append(bt)
    # v bias must broadcast along partitions (free dim = c_out): bounce DRAM
    vb_w = []
    for co in range(CT):
        w = nc.sync.dma_start(out=vbias_d[co * P : (co + 1) * P], in_=beff["v"][co])
        vb_w.append(w)
    vbias_bc = consts.tile([P, C], f32, tag="vbb", name="vbias_bc")
    vb_r = nc.sync.dma_start(out=vbias_bc, in_=vbias_d.partition_broadcast(P))
    for w in vb_w:
        add_dep_helper(vb_r.ins, w.ins, sync=True, reason="vbias dram RAW")
    # ---- scale weights in place: W_eff rows scaled by A (per c_in) ----
    for m in "qkv":
        for ci in range(CT):
            nc.vector.tensor_scalar_mul(out=wsb[m][ci], in0=wsb[m][ci], scalar1=Asb[ci])

    # ---- stage 2: K, Vt, Q convs straight from raw x ----
    q_writes = {}
    Ksb = [kvp.tile([P, N], f32r, tag=f"K{co}", name=f"K{co}") for co in range(CT)]
    Vtsb = [
        kvp.tile([P, C], f32r, tag=f"Vt{nt}", name=f"Vt{nt}") for nt in range(NKT)
    ]
    for j in range(NCH):
        xts = []
        for ci in range(CT):
            xt = xio.tile([P, NB], f32r, tag="x", name=f"x2_{ci}_{j}")
            eng = nc.sync if ci % 2 == 0 else nc.scalar
            eng.dma_start(
                out=xt, in_=x_d[ci * P : (ci + 1) * P, j * NB : (j + 1) * NB]
            )
            xts.append(xt)
        for co in range(CT):
            pk = ps_work.tile([P, NB], f32, tag="work", name=f"pk{j}_{co}")
            for ci in range(CT):
                nc.tensor.matmul(
                    pk,
                    lhsT=wsb["k"][ci][:, co * P : (co + 1) * P],
                    rhs=xts[ci],
                    start=(ci == 0), stop=(ci == CT - 1),
                )
            nc.vector.tensor_scalar_add(
                out=Ksb[co][:, j * NB : (j + 1) * NB], in0=pk, scalar1=beff["k"][co]
            )
        for sub in range(NB // P):
            pv = ps_work.tile([P, NB], f32, tag="work", name=f"pv{j}_{sub}")
            for ci in range(CT):
                nc.tensor.matmul(
                    pv,
                    lhsT=xts[ci][:, sub * P : (sub + 1) * P],
                    rhs=wsb["v"][ci],
                    start=(ci == 0), stop=(ci == CT - 1),
                )
            nc.vector.tensor_add(Vtsb[j * (NB // P) + sub], pv, vbias_bc)
        if j < NQ // NB:
            for co in range(CT):
                pq = ps_work.tile([P, NB], f32, tag="work", name=f"pq{j}_{co}")
                for ci in range(CT):
                    nc.tensor.matmul(
                        pq,
                        lhsT=wsb["q"][ci][:, co * P : (co + 1) * P],
                        rhs=xts[ci],
                        start=(ci == 0), stop=(ci == CT - 1),
                    )
                qt_sb = qs.tile([P, NB], f32r, tag="qconv", name=f"qc{j}_{co}", bufs=2)
                nc.vector.tensor_scalar_add(out=qt_sb, in0=pq, scalar1=beff["q"][co])
                qw = nc.gpsimd.dma_start(
                    out=qsc_d[co * P : (co + 1) * P, j * NB : (j + 1) * NB],
                    in_=qt_sb,
                )
                q_writes[(j, co)] = qw

    # wp loaded late: reuses wq slots (same tags)
    wsb["p"] = []
    for ci in range(CT):
        t = consts.tile([P, C], f32r, tag=f"wq{ci}", name=f"wp{ci}")
        nc.gpsimd.dma_start(out=t, in_=w_d["p"][ci * P : (ci + 1) * P, :])
        wsb["p"].append(t)

    # ---- stage 3: attention + proj per query block ----
    for qb in range(NQB):
        qtiles = []
        for ci in range(CT):
            qt_l = qs.tile([P, QBW], f32r, tag=f"qqb{ci}", name=f"qqb{qb}_{ci}", bufs=2)
            qr = nc.sync.dma_start(
                out=qt_l,
                in_=qsc_d[ci * P : (ci + 1) * P, qb * QBW : (qb + 1) * QBW],
            )
            add_dep_helper(
                qr.ins, q_writes[(qb, ci)].ins, sync=True, reason="qsc dram RAW"
            )
            qtiles.append(qt_l)
        att_ps = [
            ps_out.tile([P, QBW], f32, tag="out", name=f"attps{qb}_{co}")
            for co in range(CT)
        ]
        rs = ps_work.tile([1, QBW], f32, tag="work", name=f"rs{qb}")
        rsacc = qs.tile([P, QBW], f32, tag="rsacc", name=f"rsacc{qb}", bufs=1)
        for nt in range(NKT):
            st = ps_work.tile([P, QBW], f32, tag="work", name=f"st{qb}_{nt}")
            for ci in range(CT):
                nc.tensor.matmul(
                    st,
                    lhsT=Ksb[ci][:, nt * P : (nt + 1) * P],
                    rhs=qtiles[ci],
                    start=(ci == 0), stop=(ci == CT - 1),
                )
            pt = ptp.tile([P, QBW], f32r, tag="pt", name=f"pt{qb}_{nt}")
            nc.scalar.activation(out=pt, in_=st, func=AF.Exp, scale=ISQ)
            if nt == 0:
                nc.vector.tensor_copy(rsacc, pt)
            else:
                nc.vector.tensor_add(rsacc, rsacc, pt)
            for co in range(CT):
                nc.tensor.matmul(
                    att_ps[co],
                    lhsT=Vtsb[nt][:, co * P : (co + 1) * P],
                    rhs=pt,
                    start=(nt == 0), stop=(nt == NKT - 1),
                )
        # single f32 matmul collapses the DVE-accumulated exp sums over keys
        nc.tensor.matmul(rs, lhsT=ones_f32, rhs=rsacc, start=True, stop=True)
        rs_sb = fop.tile([1, QBW], f32, tag="fo", name=f"rssb{qb}")
        nc.vector.reciprocal(rs_sb, rs)
        rw = nc.sync.dma_start(out=rsc_d[qb : qb + 1, :], in_=rs_sb)
        rbc = consts.tile([P, QBW], f32, tag="vbb", name=f"rbc{qb}")
        rr = nc.sync.dma_start(out=rbc, in_=rsc_d[qb, :].partition_broadcast(P))
        add_dep_helper(rr.ins, rw.ins, sync=True, reason="rsc dram RAW")
        # move unnormalized attn [c, nq] to SBUF for proj rhs
        att_sb = []
        for co in range(CT):
            t = consts.tile([P, QBW], f32r, tag=f"wv{co}", name=f"attsb{qb}_{co}")
            if co % 2 == 0:
                nc.scalar.copy(out=t, in_=att_ps[co])
            else:
                nc.vector.tensor_copy(t, att_ps[co])
            att_sb.append(t)
        for co in range(CT):
            pp = ps_work.tile([P, QBW], f32, tag="work", name=f"pp{qb}_{co}")
            for ci in range(CT):
                nc.tensor.matmul(
                    pp,
                    lhsT=wsb["p"][ci][:, co * P : (co + 1) * P],
                    rhs=att_sb[ci],
                    start=(ci == 0), stop=(ci == CT - 1),
                )
            xr = xio.tile([P, QBW], f32r, tag="x", name=f"xr{qb}_{co}")
            (nc.sync if co % 2 == 0 else nc.scalar).dma_start(
                out=xr, in_=x_d[co * P : (co + 1) * P, qb * QBW : (qb + 1) * QBW]
            )
            fo = fop.tile([P, QBW], f32, tag="fo", name=f"fo{qb}_{co}")
            # fo = pp * (1/rowsum) + bp + x
            nc.vector.tensor_mul(fo, pp, rbc)
            nc.vector.tensor_scalar_add(out=fo, in0=fo, scalar1=bsb["p"][co])
            nc.vector.tensor_add(fo, fo, xr)
            (nc.scalar if co % 2 == 0 else nc.sync).dma_start(
                out=out_d[co * P : (co + 1) * P, qb * QBW : (qb + 1) * QBW], in_=fo
            )


def _build_program(reps=1):
    bass, bacc, tile, mybir, _, make_identity = _imports()
    f32 = mybir.dt.float32
    f32r = mybir.dt.float32r

    nc = bacc.Bacc("TRN2", target_bir_lowering=False, debug=False, num_devices=8)

    io = {}
    io["x"] = nc.dram_tensor("x", [C, N], f32r, kind="ExternalInput").ap()
    io["w"] = {}
    io["b"] = {}
    for m in "qkvp":
        io["w"][m] = nc.dram_tensor(f"w{m}T", [C, C], f32r, kind="ExternalInput").ap()
        io["b"][m] = nc.dram_tensor(f"b{m}", [C, 1], f32, kind="ExternalInput").ap()
    io["nw"] = nc.dram_tensor("nw", [C, 1], f32, kind="ExternalInput").ap()
    io["nb"] = nc.dram_tensor("nb", [C, 1], f32, kind="ExternalInput").ap()
    io["gm"] = nc.dram_tensor("gmask", [CT, P, G], f32, kind="ExternalInput").ap()
    io["gmT"] = nc.dram_tensor("gmaskT", [CT, G, P], f32, kind="ExternalInput").ap()
    io["out"] = nc.dram_tensor("out", [C, NQ], f32, kind="ExternalOutput").ap()
    io["qsc"] = nc.dram_tensor("qsc", [C, NQ], f32r, kind="Internal").ap()
    io["vbias"] = nc.dram_tensor("vbias", [C], f32, kind="Internal").ap()
    io["rsc"] = nc.dram_tensor("rsc", [NQB, QBW], f32, kind="Internal").ap()
    nc._io = io

    with tile.TileContext(nc) as tc, ExitStack() as ctx:
        pools = {}
        pools["consts"] = ctx.enter_context(tc.tile_pool(name="consts", bufs=1))
        pools["kv"] = ctx.enter_context(tc.tile_pool(name="kv", bufs=1))
        pools["xio"] = ctx.enter_context(tc.tile_pool(name="xio", bufs=8))
        pools["qs"] = ctx.enter_context(tc.tile_pool(name="qs", bufs=4))
        pools["ptp"] = ctx.enter_context(tc.tile_pool(name="ptp", bufs=3))
        pools["fop"] = ctx.enter_context(tc.tile_pool(name="fop", bufs=2))
        pools["stats"] = ctx.enter_context(tc.tile_pool(name="stats", bufs=1))
        pools["bstp"] = ctx.enter_context(tc.tile_pool(name="bstp", bufs=2))
        pools["ps_work"] = ctx.enter_context(
            tc.tile_pool(name="ps_work", bufs=4, space="PSUM")
        )
        pools["ps_out"] = ctx.enter_context(
            tc.tile_pool(name="ps_out", bufs=4, space="PSUM")
        )
        nc._pools = pools

        if reps > 1:
            with tc.For_i(0, reps, 1):
                _build_body(nc, tc, ctx, bass, tile, mybir)
        else:
            _build_body(nc, tc, ctx, bass, tile, mybir)

    nc.compile()
    return nc


@functools.lru_cache(maxsize=2)
def _get_nc(reps=1):
    return _build_program(reps)


def _host_inputs(x, norm_w, norm_b, q_w, q_b, k_w, k_b, v_w, v_b, proj_w, proj_b):
    """Build the 8 per-core input maps."""
    x = np.asarray(x)
    q_w, k_w, v_w, proj_w = (np.asarray(a) for a in (q_w, k_w, v_w, proj_w))
    B = x.shape[0]
    xf = np.ascontiguousarray(x.reshape(B, C, N)).astype(np.float32)
    gm = np.zeros((CT, P, G), np.float32)
    gmT = np.zeros((CT, G, P), np.float32)
    for ci in range(CT):
        for c in range(P):
            gm[ci, c, (ci * P + c) // GSZ] = 1.0 / GSZ
            gmT[ci, (ci * P + c) // GSZ, c] = 1.0
    shared = {
        "wqT": np.ascontiguousarray(q_w.T).astype(np.float32),
        "wkT": np.ascontiguousarray(k_w.T).astype(np.float32),
        "wvT": np.ascontiguousarray(v_w.T).astype(np.float32),
        "wpT": np.ascontiguousarray(proj_w.T).astype(np.float32),
        "bq": np.asarray(q_b, np.float32).reshape(C, 1),
        "bk": np.asarray(k_b, np.float32).reshape(C, 1),
        "bv": np.asarray(v_b, np.float32).reshape(C, 1),
        "bp": np.asarray(proj_b, np.float32).reshape(C, 1),
        "nw": np.asarray(norm_w, np.float32).reshape(C, 1),
        "nb": np.asarray(norm_b, np.float32).reshape(C, 1),
        "gmask": gm,
        "gmaskT": gmT,
    }
    in_maps = []
    for core in range(8):
        b, hf = core // 2, core % 2
        if hf == 0:
            xp = xf[b]
        else:
            xp = np.concatenate([xf[b, :, NQ:], xf[b, :, :NQ]], axis=1)
        in_maps.append({"x": np.ascontiguousarray(xp), **shared})
    return in_maps


def kernel(**inputs):
    _, _, _, _, run_bass_kernel_spmd, _ = _imports()
    nc = _get_nc()
    in_maps = _host_inputs(**inputs)
    res = run_bass_kernel_spmd(nc, in_maps, core_ids=list(range(8)))
    x = inputs["x"]
    B = x.shape[0]
    out = np.empty((B, C, N), np.float32)
    for core in range(8):
        b, hf = core // 2, core % 2
        out[b, :, hf * NQ : (hf + 1) * NQ] = res.results[core]["out"]
    return out.reshape(x.shape)



# revision 20
# speedup vs baseline: 1.3958x; 1.3958x over previous
"""AttentionBlock (GroupNorm -> QKV 1x1 conv -> softmax attention -> proj conv
-> residual) as a Bass/Tile kernel for 8 Trainium2 NeuronCores.

Sharding: core c handles batch b=c//2, query-half hf=c%2 (2048 of 4096 tokens).
Host permutes each core's x so its query half is always columns 0:2048 (keys are
permutation-invariant under softmax attention), making the program identical on
every core (SPMD). K and V are computed for the full 4096 tokens on both cores
of a batch (duplicated conv work, no collectives needed).

v2: every large matmul runs in fp8(e4m3) with perf_mode=DoubleRow — the PE
packs 2 contraction rows per cell (contraction dim 256 per matmul), roughly
halving PE time vs f32r. All fp8 operands live in plane-major packed tiles
[128, 2, free] where (partition, plane) = contraction index; plane writes are
contiguous slices so conv-psum evictions are plain engine copies.

GroupNorm is applied to x explicitly (H = A*x + B per channel, one
tensor_scalar per tile) instead of folding into the weights, so all conv
weights are host-packed fp8 constants. Bias algebra: bk cancels in softmax
(per-query common mode) and is dropped; bq folds into the exp bias via
kb = K^T bq (tiny DoubleRow matmuls); bv commutes through attention
(sum p =rowsum cancels after normalization) and is folded into the proj bias
ON THE HOST: bpe = proj_b + proj_w @ v_b; bp lives in the fused epilogue.

Attention: S^T = K^T Q per 128-key tile; P' = exp(ISQ*S^T + ISQ*kb - 3.0)
(the -3.0 shift cancels in normalization and keeps exp outputs ~<=20, far from
fp8's 240 overflow-to-Inf). Rowsum rides the PV matmul for free via a padded
ones column in the packed V tiles (psum row [1,512] accumulated over key
tiles). 1/rowsum is broadcast to 128 partitions with a rank-1 PE matmul
(ones_row x rs) - no DRAM round trips anywhere; K/V/Q/P stay resident in SBUF
(fp8 makes them ~10x smaller).
"""

import functools
import sys
from contextlib import ExitStack

import numpy as np


def _imports():
    try:
        import concourse.bass  # noqa: F401
    except ImportError:
        sys.path.insert(0, "/opt/trn_rl_repo")
    import concourse.bass as bass
    import concourse.tile as tile
    from concourse import bacc, mybir
    from concourse.bass_utils import run_bass_kernel_spmd

    return bass, bacc, tile, mybir, run_bass_kernel_spmd


P = 128          # partitions
C = 512          # channels
CT = C // P      # 4 channel tiles
N = 4096         # tokens per batch (64*64)
NQ = 2048        # queries per core
NB = 512         # n-chunk width
NCH = N // NB    # 8 n-chunks
QBW = 512        # query block width
NQB = NQ // QBW  # 4 query blocks
NKT = N // P     # 32 key tiles
NPAIR = NKT // 2  # 16 key-tile pairs (DoubleRow)
G = 32           # groups
GSZ = C // G     # 16 channels per group
EPS = 1e-5
ISQ = 1.0 / float(np.sqrt(C))
SHIFT = 3.0      # exp(s - SHIFT): cancels in softmax, avoids fp8 overflow
VW = 528         # padded Vt8 width: col 512 = ones (rowsum), 16-aligned
CBW = 656        # f32 const-buffer columns (gm|nw|nb|bpe|gmT|bq)


def _build_body(nc, tc, ctx, bass, tile, mybir):
    import os
    _skip = set(os.environ.get("BISECT_SKIP", "").split(","))
    f32 = mybir.dt.float32
    f32r = mybir.dt.float32r
    fp8 = mybir.dt.float8e4
    AF = mybir.ActivationFunctionType
    OP = mybir.AluOpType
    DR = mybir.MatmulPerfMode.DoubleRow

    x_d = nc._io["x"]
    cbuf_d = nc._io["cbuf"]
    wall_d = nc._io["wall"]
    out_d = nc._io["out"]
    pools = nc._pools
    consts = pools["consts"]
    xres = pools["xres"]
    h8p = pools["h8"]
    kv8 = pools["kv8"]
    p8p = pools["p8"]
    wk = pools["work"]
    stats = pools["stats"]
    bstp = pools["bstp"]
    ps_work = pools["ps_work"]
    ps_att = pools["ps_att"]
    ps_small = pools["ps_small"]

    # ---- constants ----
    # DMA issue costs ~1.26us of queue time per descriptor regardless of
    # size, so all constants arrive in TWO host-packed transfers: a f32
    # "cbuf" (masks, norm affine, proj bias, bq) and an fp8 weight "wall".
    # Everything else is an AP view into those two resident tiles.
    cb = consts.tile([P, CBW], f32, tag="cbuf", name="cbuf")
    nc.sync.dma_start(out=cb, in_=cbuf_d)
    wall = consts.tile([P, 4, 2, 2, C], fp8, tag="wall", name="wall")
    nc.scalar.dma_start(out=wall, in_=wall_d)
    gmsb = [cb[:, 32 * ci : 32 * ci + 32] for ci in range(CT)]
    nwsb = [cb[:, 128 + ci : 129 + ci] for ci in range(CT)]
    nbsb = [cb[:, 132 + ci : 133 + ci] for ci in range(CT)]
    bpesb = [cb[:, 136 + ci : 137 + ci] for ci in range(CT)]
    gmTsb = [cb[0:G, 140 + P * ci : 140 + P * (ci + 1)] for ci in range(CT)]
    w8 = {m: [wall[:, mi, pt] for pt in range(2)] for mi, m in enumerate("qkvp")}
    # memset cannot target f32r: set via f32 then tensor_copy
    ones_row_f = consts.tile([1, P], f32, tag="ones_row_f", name="ones_row_f")
    nc.vector.memset(ones_row_f, 1.0)
    ones_row = consts.tile([1, P], f32r, tag="ones_row", name="ones_row")
    nc.vector.tensor_copy(ones_row, ones_row_f)
    eps32 = consts.tile([G, 1], f32, tag="eps32", name="eps32")
    nc.vector.memset(eps32, EPS)
    # bq8 packed fp8 [P, 2, 16] per pair tile, built from cbuf columns
    ones16 = consts.tile([P, 16], f32, tag="ones16", name="ones16")
    nc.vector.memset(ones16, 1.0)
    bq8 = []
    for pt in range(2):
        t = consts.tile([P, 2, 16], fp8, tag=f"bq{pt}", name=f"bq8{pt}")
        for pl in range(2):
            nc.vector.tensor_scalar(
                out=t[:, pl, :], in0=ones16,
                scalar1=cb[:, 652 + 2 * pt + pl : 653 + 2 * pt + pl],
                scalar2=None, op0=OP.mult,
            )
        bq8.append(t)

    # ---- resident SBUF state ----
    # x is loaded once as 16 [128, 1024] f32 "superchunk" tiles (ci, j2) and
    # stays resident: stats, normalization, and the stage-3 residual all read
    # it from SBUF. fp8 K/Q/V/P tiles are small enough to keep resident too.
    xt = {}
    K8 = [kv8.tile([P, 2, N], fp8, tag=f"K8{pt}", name=f"K8{pt}") for pt in range(2)]
    Q8 = [kv8.tile([P, 2, NQ], fp8, tag=f"Q8{pt}", name=f"Q8{pt}") for pt in range(2)]
    Vt8 = [
        kv8.tile([P, 2, VW], fp8, tag=f"Vt8{t}", name=f"Vt8{t}") for t in range(NPAIR)
    ]
    for t in range(NPAIR):
        nc.vector.memset(Vt8[t][:, :, C:VW], 0.0)
        nc.vector.memset(Vt8[t][:, :, C : C + 1], 1.0)

    # ---- stage 1: GroupNorm statistics (streaming pass over x) ----
    NSC = NCH // 2  # 4 superchunks of 1024 tokens
    mv = []
    for ci in range(CT):
        bst = bstp.tile([P, NCH, 6], f32, tag=f"bst{ci}", name=f"bst{ci}")
        for j2 in range(NSC):
            t = xres.tile([P, 2 * NB], f32, tag=f"x{ci}_{j2}", name=f"x{ci}_{j2}")
            eng = nc.sync if (ci * NSC + j2) % 2 == 0 else nc.scalar
            eng.dma_start(
                out=t,
                in_=x_d[ci * P : (ci + 1) * P, j2 * 2 * NB : (j2 + 1) * 2 * NB],
            )
            xt[(ci, j2)] = t
            # bn_stats free dim is HW-capped at 512: two per superchunk
            nc.vector.bn_stats(out=bst[:, 2 * j2, :], in_=t[:, 0:NB])
            nc.vector.bn_stats(out=bst[:, 2 * j2 + 1, :], in_=t[:, NB : 2 * NB])
        m = stats.tile([P, 2], f32, tag=f"mv{ci}", name=f"mv{ci}")
        nc.vector.bn_aggr(out=m, in_=bst)
        # m[:,1] := var + mean^2 = E[x^2]
        tmp = stats.tile([P, 1], f32, tag=f"tmp{ci}", name=f"tmp{ci}")
        nc.vector.tensor_mul(tmp, m[:, 0:1], m[:, 0:1])
        nc.vector.tensor_add(m[:, 1:2], m[:, 1:2], tmp)
        mv.append(m)
    ps_g = ps_small.tile([G, 2], f32, tag="small", name="psg")
    for ci in range(CT):
        nc.tensor.matmul(
            ps_g, lhsT=gmsb[ci], rhs=mv[ci], start=(ci == 0), stop=(ci == CT - 1)
        )
    gs = stats.tile([G, 2], f32, tag="gs", name="gs")  # [gmean, gE[x^2]]
    nc.vector.tensor_copy(gs, ps_g)
    gvar = stats.tile([G, 1], f32, tag="gvar", name="gvar")
    nc.vector.tensor_mul(gvar, gs[:, 0:1], gs[:, 0:1])
    nc.vector.tensor_sub(gvar, gs[:, 1:2], gvar)
    grstd = stats.tile([G, 1], f32, tag="grstd", name="grstd")
    nc.scalar.activation(out=grstd, in_=gvar, func=AF.Sqrt, bias=eps32, scale=1.0)
    nc.vector.reciprocal(grstd, grstd)
    gsr = stats.tile([G, 2], f32, tag="gsr", name="gsr")  # [gmean, grstd]
    nc.vector.tensor_copy(gsr[:, 0:1], gs[:, 0:1])
    nc.vector.tensor_copy(gsr[:, 1:2], grstd)
    Asb, Bsb = [], []
    for ci in range(CT):
        # independent per-ci chains: split across DVE/Pool to shorten the
        # serial small-op tail between the stats barrier and the first conv
        eng = nc.vector if ci % 2 == 0 else nc.gpsimd
        mrps = ps_small.tile([P, 2], f32, tag="small", name=f"mrps{ci}")
        nc.tensor.matmul(mrps, lhsT=gmTsb[ci], rhs=gsr, start=True, stop=True)
        mr = stats.tile([P, 2], f32, tag=f"mr{ci}", name=f"mr{ci}")
        nc.vector.tensor_copy(mr, mrps)
        a = stats.tile([P, 1], f32, tag=f"A{ci}", name=f"A{ci}")
        eng.tensor_mul(a, mr[:, 1:2], nwsb[ci])
        bb = stats.tile([P, 1], f32, tag=f"Bf{ci}", name=f"Bf{ci}")
        eng.tensor_mul(bb, mr[:, 0:1], a)
        eng.tensor_sub(bb, nbsb[ci], bb)
        Asb.append(a)
        Bsb.append(bb)

    eb = stats.tile([P, NKT], f32, tag="eb", name="eb")  # exp bias per key tile

    # ---- stage 2: normalize+pack H8, then K/V/Q convs (all fp8 DoubleRow) ----
    # h8 superchunk tiles [128, 2, 1024]; conv rhs slices are [128, 2, 512].
    # psum evictions round-robin over DVE/Act/Pool; V-conv psum borrows the
    # (stage-3-only) ps_att pool to relieve ps_work slot pressure.
    ev = {"i": 0}
    ev_engs = [nc.vector, nc.scalar]  # GPSIMD cannot access PSUM

    def evict(dst, src):
        e = ev_engs[ev["i"] % 2]
        ev["i"] += 1
        if e is nc.scalar:
            e.copy(out=dst, in_=src)
        else:
            e.tensor_copy(dst, src)

    for j2 in range(NSC):
        # exp-bias folds lag one superchunk so the tiny PE matmuls never
        # stall on this superchunk's K-evictions: eb = ISQ*(K^T bq) - SHIFT
        lo = 8 * (j2 - 1) if j2 > 0 else None
        if "kbp" in _skip:
            if j2 == 0:
                nc.vector.memset(eb, -SHIFT)
            lo = None
        for nt in ([] if lo is None else range(lo, lo + 8)):
            kbp = ps_small.tile([P, 16], f32, tag="small", name=f"kbp{nt}")
            nc.tensor.matmul(
                kbp, lhsT=K8[0][:, :, nt * P : (nt + 1) * P], rhs=bq8[0],
                start=True, stop=False, perf_mode=DR,
            )
            nc.tensor.matmul(
                kbp, lhsT=K8[1][:, :, nt * P : (nt + 1) * P], rhs=bq8[1],
                start=False, stop=True, perf_mode=DR,
            )
            nc.vector.tensor_scalar(
                out=eb[:, nt : nt + 1], in0=kbp[:, 0:1],
                scalar1=ISQ, scalar2=-SHIFT, op0=OP.mult, op1=OP.add,
            )

        h8 = [
            h8p.tile([P, 2, 2 * NB], fp8, tag=f"h8{pt}", name=f"h8{pt}_{j2}")
            for pt in range(2)
        ]
        for ci in range(CT):
            pt, pl = divmod(ci, 2)
            eng = nc.vector if ci % 2 == 0 else nc.gpsimd
            eng.tensor_scalar(
                out=h8[pt][:, pl, :],
                in0=xt[(ci, j2)],
                scalar1=Asb[ci],
                scalar2=Bsb[ci],
                op0=OP.mult,
                op1=OP.add,
            )
        for jj in range(2):
            j = 2 * j2 + jj
            h8s = [h8[pt][:, :, jj * NB : (jj + 1) * NB] for pt in range(2)]
            # K conv: [c_out, tokens]
            for co in range(CT):
                pk = ps_work.tile([P, NB], f32, tag="work", name=f"pk{j}_{co}")
                nc.tensor.matmul(
                    pk, lhsT=w8["k"][0][:, :, co * P : (co + 1) * P], rhs=h8s[0],
                    start=True, stop=False, perf_mode=DR,
                )
                nc.tensor.matmul(
                    pk, lhsT=w8["k"][1][:, :, co * P : (co + 1) * P], rhs=h8s[1],
                    start=False, stop=True, perf_mode=DR,
                )
                pt, pl = divmod(co, 2)
                evict(K8[pt][:, pl, j * NB : (j + 1) * NB], pk)
            # V conv: [tokens, c_out]
            for sub in range(NB // P):
                sg = j * (NB // P) + sub
                t, pl = divmod(sg, 2)
                pv = ps_att.tile([P, NB], f32, tag="att", name=f"pv{j}_{sub}")
                nc.tensor.matmul(
                    pv, lhsT=h8s[0][:, :, sub * P : (sub + 1) * P], rhs=w8["v"][0],
                    start=True, stop=False, perf_mode=DR,
                )
                nc.tensor.matmul(
                    pv, lhsT=h8s[1][:, :, sub * P : (sub + 1) * P], rhs=w8["v"][1],
                    start=False, stop=True, perf_mode=DR,
                )
                evict(Vt8[t][:, pl, 0:C], pv)
            # Q conv (first NQ tokens only)
            if j < NQ // NB:
                for co in range(CT):
                    pq = ps_work.tile([P, NB], f32, tag="work", name=f"pq{j}_{co}")
                    nc.tensor.matmul(
                        pq, lhsT=w8["q"][0][:, :, co * P : (co + 1) * P], rhs=h8s[0],
                        start=True, stop=False, perf_mode=DR,
                    )
                    nc.tensor.matmul(
                        pq, lhsT=w8["q"][1][:, :, co * P : (co + 1) * P], rhs=h8s[1],
                        start=False, stop=True, perf_mode=DR,
                    )
                    pt, pl = divmod(co, 2)
                    evict(Q8[pt][:, pl, j * NB : (j + 1) * NB], pq)

    for nt in ([] if "kbp" in _skip else range(8 * (NSC - 1), NKT)):
        kbp = ps_small.tile([P, 16], f32, tag="small", name=f"kbp{nt}")
        nc.tensor.matmul(
            kbp, lhsT=K8[0][:, :, nt * P : (nt + 1) * P], rhs=bq8[0],
            start=True, stop=False, perf_mode=DR,
        )
        nc.tensor.matmul(
            kbp, lhsT=K8[1][:, :, nt * P : (nt + 1) * P], rhs=bq8[1],
            start=False, stop=True, perf_mode=DR,
        )
        nc.vector.tensor_scalar(
            out=eb[:, nt : nt + 1], in0=kbp[:, 0:1],
            scalar1=ISQ, scalar2=-SHIFT, op0=OP.mult, op1=OP.add,
        )

    # ---- stage 3: attention + proj per query block ----
    # Software pipeline: PV lags exp by 2 key-tile pairs so the PE never waits
    # on a fresh exp except at the very last pair, and the previous qb's
    # epilogue (1/rowsum -> normalize -> proj -> residual -> store) is spread
    # over the first ~8 S/exp slots of the current qb.
    def _epi_recip(qb, rs):
        rs_sb = wk.tile([1, QBW], f32r, tag="rssb", name=f"rssb{qb}", bufs=2)
        with nc.allow_low_precision(reason="f32r == f32 bits; PE bcast operand"):
            nc.vector.reciprocal(rs_sb, rs)
        return rs_sb

    def _epi_rbc(qb, rs_sb):
        rbc = ps_small.tile([P, QBW], f32, tag="small", name=f"rbc{qb}")
        if "rbc" in _skip:
            nc.vector.memset(rbc, 1.0)
        else:
            nc.tensor.matmul(rbc, lhsT=ones_row, rhs=rs_sb, start=True, stop=True)
        return rbc

    def _epi_muls(qb, att_ps, rbc):
        # DVE can read only one PSUM operand per op: land rbc in SBUF first
        rbc_sb = wk.tile([P, QBW], f32, tag="rbcsb", name=f"rbcsb{qb}", bufs=2)
        nc.vector.tensor_copy(rbc_sb, rbc)
        att8 = [
            wk.tile([P, 2, QBW], fp8, tag=f"att8{pt}", name=f"att8{qb}_{pt}", bufs=2)
            for pt in range(2)
        ]
        for co in range(CT):
            pt, pl = divmod(co, 2)
            nc.vector.tensor_mul(att8[pt][:, pl, :], att_ps[co], rbc_sb)
        return att8

    def _epi_proj1(qb, att8, co, fo):
        pp = ps_work.tile([P, QBW], f32, tag="work", name=f"pp{qb}_{co}")
        nc.tensor.matmul(
            pp, lhsT=w8["p"][0][:, :, co * P : (co + 1) * P], rhs=att8[0],
            start=True, stop=False, perf_mode=DR,
        )
        nc.tensor.matmul(
            pp, lhsT=w8["p"][1][:, :, co * P : (co + 1) * P], rhs=att8[1],
            start=False, stop=True, perf_mode=DR,
        )
        # fo = (pp + bpe) + x   (proj bias incl. host-folded Wp@bv; psum
        # input so DVE only - GPSIMD cannot access PSUM)
        nc.vector.scalar_tensor_tensor(
            out=fo[:, co, :], in0=pp, scalar=bpesb[co], in1=xt[(co, qb // 2)][
                :, (qb % 2) * QBW : (qb % 2 + 1) * QBW
            ],
            op0=OP.add, op1=OP.add,
        )
        if co == CT - 1:
            # one store per qb on the sync queue (a DMA issue stalls the
            # issuing queue ~1.26us; Act must keep streaming exps); the last
            # qb splits halves across both queues to shorten the tail
            dst = out_d[:, :, qb * QBW : (qb + 1) * QBW]
            if qb == NQB - 1:
                nc.sync.dma_start(out=dst[:, 0:2, :], in_=fo[:, 0:2, :])
                nc.scalar.dma_start(out=dst[:, 2:4, :], in_=fo[:, 2:4, :])
            else:
                nc.sync.dma_start(out=dst, in_=fo)

    def _pv(qb, att_ps, rs, Vt8t, p8t, t):
        for co in range(CT):
            nc.tensor.matmul(
                att_ps[co], lhsT=Vt8t[:, :, co * P : (co + 1) * P], rhs=p8t,
                start=(t == 0), stop=(t == NPAIR - 1), perf_mode=DR,
            )
        if "rowsum" in _skip:
            if t == 0:
                nc.vector.memset(rs, 1.0)
        else:
            nc.tensor.matmul(
                rs, lhsT=Vt8t[:, :, C : C + 1], rhs=p8t,
                start=(t == 0), stop=(t == NPAIR - 1), perf_mode=DR,
            )

    prev = None  # (qb, att_ps, rs) awaiting epilogue
    for qb in range(NQB):
        q8s = [Q8[pt][:, :, qb * QBW : (qb + 1) * QBW] for pt in range(2)]
        att_ps = None
        rs = None
        p8t = None
        p8tiles = {}
        e_rssb = e_rbc = e_att8 = None
        e_fo = None
        for nt in range(NKT):
            t, pl = divmod(nt, 2)
            st = ps_work.tile([P, QBW], f32, tag="work", name=f"st{qb}_{nt}")
            nc.tensor.matmul(
                st, lhsT=K8[0][:, :, nt * P : (nt + 1) * P], rhs=q8s[0],
                start=True, stop=False, perf_mode=DR,
            )
            nc.tensor.matmul(
                st, lhsT=K8[1][:, :, nt * P : (nt + 1) * P], rhs=q8s[1],
                start=False, stop=True, perf_mode=DR,
            )
            if pl == 0:
                p8t = p8p.tile(
                    [P, 2, QBW], fp8, tag="p8", name=f"p8_{qb}_{t}", bufs=4
                )
                p8tiles[t] = p8t
            nc.scalar.activation(
                out=p8t[:, pl, :], in_=st, func=AF.Exp,
                bias=eb[:, nt : nt + 1], scale=ISQ,
            )
            if prev is not None:
                if nt == 0:
                    e_rssb = _epi_recip(prev[0], prev[2])
                elif nt == 1:
                    e_rbc = _epi_rbc(prev[0], e_rssb)
                elif nt == 2:
                    e_att8 = _epi_muls(prev[0], prev[1], e_rbc)
                elif 4 <= nt <= 7:
                    if nt == 4:
                        e_fo = wk.tile(
                            [P, CT, QBW], f32, tag="fo", name=f"fo{prev[0]}", bufs=2
                        )
                    _epi_proj1(prev[0], e_att8, nt - 4, e_fo)
                    if nt == 7:
                        prev = None
            if nt == 3:
                att_ps = [
                    ps_att.tile([P, QBW], f32, tag="att", name=f"attps{qb}_{co}")
                    for co in range(CT)
                ]
                rs = ps_small.tile([1, QBW], f32, tag="small", name=f"rs{qb}")
            if nt >= 3 and pl == 1:
                tl = (nt - 3) // 2  # lagged pair: 0 at nt3, .., 14 at nt31
                _pv(qb, att_ps, rs, Vt8[tl], p8tiles.pop(tl), tl)
        _pv(qb, att_ps, rs, Vt8[NPAIR - 1], p8tiles.pop(NPAIR - 1), NPAIR - 1)
        prev = (qb, att_ps, rs)
    e_rssb = _epi_recip(prev[0], prev[2])
    e_rbc = _epi_rbc(prev[0], e_rssb)
    e_att8 = _epi_muls(prev[0], prev[1], e_rbc)
    e_fo = wk.tile([P, CT, QBW], f32, tag="fo", name=f"fo{prev[0]}", bufs=2)
    for co in range(CT):
        _epi_proj1(prev[0], e_att8, co, e_fo)


def _build_program(reps=1):
    bass, bacc, tile, mybir, _ = _imports()
    f32 = mybir.dt.float32
    fp8 = mybir.dt.float8e4

    nc = bacc.Bacc("TRN2", target_bir_lowering=False, debug=False, num_devices=8)

    io = {}
    io["x"] = nc.dram_tensor("x", [C, N], f32, kind="ExternalInput").ap()
    io["cbuf"] = nc.dram_tensor("cbuf", [P, CBW], f32, kind="ExternalInput").ap()
    io["wall"] = nc.dram_tensor(
        "wall", [P, 4, 2, 2, C], fp8, kind="ExternalInput"
    ).ap()
    io["out"] = nc.dram_tensor("out", [P, CT, NQ], f32, kind="ExternalOutput").ap()
    nc._io = io

    with tile.TileContext(nc) as tc, ExitStack() as ctx:
        pools = {}
        pools["consts"] = ctx.enter_context(tc.tile_pool(name="consts", bufs=1))
        pools["xres"] = ctx.enter_context(tc.tile_pool(name="xres", bufs=1))
        pools["h8"] = ctx.enter_context(tc.tile_pool(name="h8", bufs=3))
        pools["kv8"] = ctx.enter_context(tc.tile_pool(name="kv8", bufs=1))
        pools["p8"] = ctx.enter_context(tc.tile_pool(name="p8", bufs=4))
        pools["work"] = ctx.enter_context(tc.tile_pool(name="work", bufs=2))
        pools["stats"] = ctx.enter_context(tc.tile_pool(name="stats", bufs=1))
        pools["bstp"] = ctx.enter_context(tc.tile_pool(name="bstp", bufs=1))
        pools["ps_work"] = ctx.enter_context(
            tc.tile_pool(name="ps_work", bufs=3, space="PSUM")
        )
        pools["ps_att"] = ctx.enter_context(
            tc.tile_pool(name="ps_att", bufs=4, space="PSUM")
        )
        pools["ps_small"] = ctx.enter_context(
            tc.tile_pool(name="ps_small", bufs=1, space="PSUM")
        )
        nc._pools = pools

        if reps > 1:
            with tc.For_i(0, reps, 1):
                _build_body(nc, tc, ctx, bass, tile, mybir)
        else:
            _build_body(nc, tc, ctx, bass, tile, mybir)

    nc.compile()
    return nc


@functools.lru_cache(maxsize=2)
def _get_nc(reps=1):
    return _build_program(reps)


def _pack_w8(w, e4):
    """[O, C] conv weight -> [2, 128, 2, C] fp8 lhsT pack (plane-major pairs).

    (pt, p, j, o): input channel c = pt*256 + j*128 + p, output channel o.
    """
    wT = np.ascontiguousarray(np.asarray(w, np.float32).T)  # [c_in, c_out]
    return np.ascontiguousarray(
        wT.reshape(2, 2, P, C).transpose(0, 2, 1, 3)
    ).astype(e4)


def _host_inputs(x, norm_w, norm_b, q_w, q_b, k_w, k_b, v_w, v_b, proj_w, proj_b):
    """Build the 8 per-core input maps."""
    import ml_dtypes

    e4 = ml_dtypes.float8_e4m3
    x = np.asarray(x)
    B = x.shape[0]
    xf = np.ascontiguousarray(x.reshape(B, C, N)).astype(np.float32)
    # f32 const buffer: gm[0:128] | nw[128:132] | nb[132:136] | bpe[136:140]
    # | gmT[140:652] (on partitions 0:32) | bq[652:656]
    cbuf = np.zeros((P, CBW), np.float32)
    for ci in range(CT):
        for c in range(P):
            cbuf[c, 32 * ci + (ci * P + c) // GSZ] = 1.0 / GSZ
            cbuf[(ci * P + c) // GSZ, 140 + P * ci + c] = 1.0
    cbuf[:, 128:132] = np.asarray(norm_w, np.float32).reshape(CT, P).T
    cbuf[:, 132:136] = np.asarray(norm_b, np.float32).reshape(CT, P).T
    bpe = np.asarray(proj_b, np.float32) + np.asarray(
        proj_w, np.float32
    ) @ np.asarray(v_b, np.float32)
    cbuf[:, 136:140] = bpe.reshape(CT, P).T
    # bq columns: (pt, pl) -> channels pt*256 + pl*128 + p
    cbuf[:, 652:656] = np.asarray(q_b, np.float32).reshape(2, 2, P).reshape(4, P).T
    # fp8 weight wall [P, m, pt, pl, c_out], m order q,k,v,p
    wall = np.zeros((P, 4, 2, 2, C), np.float32)
    for mi, w in enumerate((q_w, k_w, v_w, proj_w)):
        wT = np.ascontiguousarray(np.asarray(w, np.float32).T)  # [c_in, c_out]
        wall[:, mi] = wT.reshape(2, 2, P, C).transpose(2, 0, 1, 3)
    wall8 = np.ascontiguousarray(wall).astype(e4)
    shared = {"cbuf": cbuf, "wall": wall8}
    in_maps = []
    for core in range(8):
        b, hf = core // 2, core % 2
        if hf == 0:
            xp = xf[b]
        else:
            xp = np.concatenate([xf[b, :, NQ:], xf[b, :, :NQ]], axis=1)
        in_maps.append({"x": np.ascontiguousarray(xp), **shared})
    return in_maps


def kernel(**inputs):
    _, _, _, _, run_bass_kernel_spmd = _imports()
    nc = _get_nc()
    in_maps = _host_inputs(**inputs)
    res = run_bass_kernel_spmd(nc, in_maps, core_ids=list(range(8)))
    x = inputs["x"]
    B = x.shape[0]
    out = np.empty((B, C, N), np.float32)
    for core in range(8):
        b, hf = core // 2, core % 2
        # device out is [P, CT, NQ]: channel c = co*128 + p
        arr = np.asarray(res.results[core]["out"])
        out[b, :, hf * NQ : (hf + 1) * NQ] = arr.transpose(1, 0, 2).reshape(C, NQ)
    return out.reshape(x.shape)


# revision 23
# speedup vs baseline: 1.6455x; 1.1789x over previous
"""AttentionBlock (GroupNorm -> QKV 1x1 conv -> softmax attention -> proj conv
-> residual) as a Bass/Tile kernel for 8 Trainium2 NeuronCores.

Sharding: core c handles batch b=c//2, query-half hf=c%2 (2048 of 4096 tokens).
Host permutes each core's x so its query half is always columns 0:2048 (keys are
permutation-invariant under softmax attention), making the program identical on
every core (SPMD). K and V are computed for the full 4096 tokens on both cores
of a batch (duplicated conv work, no collectives needed).

v2: every large matmul runs in fp8(e4m3) with perf_mode=DoubleRow — the PE
packs 2 contraction rows per cell (contraction dim 256 per matmul), roughly
halving PE time vs f32r. All fp8 operands live in plane-major packed tiles
[128, 2, free] where (partition, plane) = contraction index; plane writes are
contiguous slices so conv-psum evictions are plain engine copies.

GroupNorm is applied to x explicitly (H = A*x + B per channel, one
tensor_scalar per tile) instead of folding into the weights, so all conv
weights are host-packed fp8 constants. Bias algebra: bk cancels in softmax
(per-query common mode) and is dropped; bq folds into the exp bias via
kb = K^T bq (tiny DoubleRow matmuls); bv commutes through attention
(sum p =rowsum cancels after normalization) and is folded into the proj bias
ON THE HOST: bpe = proj_b + proj_w @ v_b; bp lives in the fused epilogue.

Attention: S^T = K^T Q per 128-key tile; P' = exp(ISQ*S^T + ISQ*kb - 3.0)
(the -3.0 shift cancels in normalization and keeps exp outputs ~<=20, far from
fp8's 240 overflow-to-Inf). Rowsum rides the PV matmul for free via a padded
ones column in the packed V tiles (psum row [1,512] accumulated over key
tiles). 1/rowsum is broadcast to 128 partitions with a rank-1 PE matmul
(ones_row x rs) - no DRAM round trips anywhere; K/V/Q/P stay resident in SBUF
(fp8 makes them ~10x smaller).
"""

import functools
import sys
from contextlib import ExitStack

import numpy as np


def _imports():
    try:
        import concourse.bass  # noqa: F401
    except ImportError:
        sys.path.insert(0, "/opt/trn_rl_repo")
    import concourse.bass as bass
    import concourse.tile as tile
    from concourse import bacc, mybir
    from concourse.bass_utils import run_bass_kernel_spmd

    return bass, bacc, tile, mybir, run_bass_kernel_spmd


P = 128          # partitions
C = 512          # channels
CT = C // P      # 4 channel tiles
N = 4096         # tokens per batch (64*64)
NQ = 2048        # queries per core
NB = 512         # n-chunk width
NCH = N // NB    # 8 n-chunks
QBW = 512        # query block width
NQB = NQ // QBW  # 4 query blocks
NKT = N // P     # 32 key tiles
NPAIR = NKT // 2  # 16 key-tile pairs (DoubleRow)
G = 32           # groups
GSZ = C // G     # 16 channels per group
EPS = 1e-5
ISQ = 1.0 / float(np.sqrt(C))
SHIFT = 3.0      # exp(s - SHIFT): cancels in softmax, avoids fp8 overflow
VW = 528         # padded Vt8 width: col 512 = ones (rowsum), 16-aligned
CBW = 656        # f32 const-buffer columns (gm|nw|nb|bpe|gmT|bq)


def _build_body(nc, tc, ctx, bass, tile, mybir):
    import os
    _skip = set(os.environ.get("BISECT_SKIP", "").split(","))
    f32 = mybir.dt.float32
    f32r = mybir.dt.float32r
    fp8 = mybir.dt.float8e4
    AF = mybir.ActivationFunctionType
    OP = mybir.AluOpType
    DR = mybir.MatmulPerfMode.DoubleRow

    x_d = nc._io["x"]
    cbuf_d = nc._io["cbuf"]
    wall_d = nc._io["wall"]
    out_d = nc._io["out"]
    pools = nc._pools
    consts = pools["consts"]
    xres = pools["xres"]
    h8p = pools["h8"]
    kv8 = pools["kv8"]
    p8p = pools["p8"]
    wk = pools["work"]
    stats = pools["stats"]
    bstp = pools["bstp"]
    ps_work = pools["ps_work"]
    ps_att = pools["ps_att"]
    ps_small = pools["ps_small"]

    # ---- constants ----
    # DMA issue costs ~1.26us of queue time per descriptor regardless of
    # size, so all constants arrive in TWO host-packed transfers: a f32
    # "cbuf" (masks, norm affine, proj bias, bq) and an fp8 weight "wall".
    # Everything else is an AP view into those two resident tiles.
    cb = consts.tile([P, CBW], f32, tag="cbuf", name="cbuf")
    nc.sync.dma_start(out=cb, in_=cbuf_d)
    wall = consts.tile([P, 4, 2, 2, C], fp8, tag="wall", name="wall")
    nc.scalar.dma_start(out=wall, in_=wall_d)
    gmsb = [cb[:, 32 * ci : 32 * ci + 32] for ci in range(CT)]
    nwsb = [cb[:, 128 + ci : 129 + ci] for ci in range(CT)]
    nbsb = [cb[:, 132 + ci : 133 + ci] for ci in range(CT)]
    bpesb = [cb[:, 136 + ci : 137 + ci] for ci in range(CT)]
    gmTsb = [cb[0:G, 140 + P * ci : 140 + P * (ci + 1)] for ci in range(CT)]
    w8 = {m: [wall[:, mi, pt] for pt in range(2)] for mi, m in enumerate("qkvp")}
    # memset cannot target f32r: set via f32 then tensor_copy
    ones_row_f = consts.tile([1, P], f32, tag="ones_row_f", name="ones_row_f")
    nc.vector.memset(ones_row_f, 1.0)
    ones_row = consts.tile([1, P], f32r, tag="ones_row", name="ones_row")
    nc.vector.tensor_copy(ones_row, ones_row_f)
    ones_colf = consts.tile([P, 1], f32, tag="ones_colf", name="ones_colf")
    nc.vector.memset(ones_colf, 1.0)
    ones_col = consts.tile([P, 1], f32r, tag="ones_col", name="ones_col")
    nc.vector.tensor_copy(ones_col, ones_colf)
    nshift = consts.tile([P, 1], f32, tag="nshift", name="nshift")
    nc.vector.memset(nshift, -SHIFT)
    eps32 = consts.tile([G, 1], f32, tag="eps32", name="eps32")
    nc.vector.memset(eps32, EPS)
    # bq rides the Q-conv eviction as a per-partition scalar add
    bqcol = [cb[:, 652 + co : 653 + co] for co in range(CT)]

    # ---- resident SBUF state ----
    # x is loaded once as 16 [128, 1024] f32 "superchunk" tiles (ci, j2) and
    # stays resident: stats, normalization, and the stage-3 residual all read
    # it from SBUF. fp8 K/Q/V/P tiles are small enough to keep resident too.
    xt = {}
    K8 = [kv8.tile([P, 2, N], fp8, tag=f"K8{pt}", name=f"K8{pt}") for pt in range(2)]
    Q8 = [kv8.tile([P, 2, NQ], fp8, tag=f"Q8{pt}", name=f"Q8{pt}") for pt in range(2)]
    Vt8 = [
        kv8.tile([P, 2, C], fp8, tag=f"Vt8{t}", name=f"Vt8{t}") for t in range(NPAIR)
    ]

    # ---- stage 1: GroupNorm statistics (streaming pass over x) ----
    NSC = NCH // 2  # 4 superchunks of 1024 tokens
    mv = []
    for ci in range(CT):
        bst = bstp.tile([P, NCH, 6], f32, tag=f"bst{ci}", name=f"bst{ci}")
        for j2 in range(NSC):
            t = xres.tile([P, 2 * NB], f32, tag=f"x{ci}_{j2}", name=f"x{ci}_{j2}")
            eng = nc.sync if (ci * NSC + j2) % 2 == 0 else nc.scalar
            eng.dma_start(
                out=t,
                in_=x_d[ci * P : (ci + 1) * P, j2 * 2 * NB : (j2 + 1) * 2 * NB],
            )
            xt[(ci, j2)] = t
            # bn_stats free dim is HW-capped at 512: two per superchunk
            nc.vector.bn_stats(out=bst[:, 2 * j2, :], in_=t[:, 0:NB])
            nc.vector.bn_stats(out=bst[:, 2 * j2 + 1, :], in_=t[:, NB : 2 * NB])
        m = stats.tile([P, 2], f32, tag=f"mv{ci}", name=f"mv{ci}")
        nc.vector.bn_aggr(out=m, in_=bst)
        # m[:,1] := var + mean^2 = E[x^2]
        tmp = stats.tile([P, 1], f32, tag=f"tmp{ci}", name=f"tmp{ci}")
        nc.vector.tensor_mul(tmp, m[:, 0:1], m[:, 0:1])
        nc.vector.tensor_add(m[:, 1:2], m[:, 1:2], tmp)
        mv.append(m)
    ps_g = ps_small.tile([G, 2], f32, tag="small", name="psg")
    for ci in range(CT):
        nc.tensor.matmul(
            ps_g, lhsT=gmsb[ci], rhs=mv[ci], start=(ci == 0), stop=(ci == CT - 1)
        )
    gs = stats.tile([G, 2], f32, tag="gs", name="gs")  # [gmean, gE[x^2]]
    nc.vector.tensor_copy(gs, ps_g)
    gvar = stats.tile([G, 1], f32, tag="gvar", name="gvar")
    nc.vector.tensor_mul(gvar, gs[:, 0:1], gs[:, 0:1])
    nc.vector.tensor_sub(gvar, gs[:, 1:2], gvar)
    grstd = stats.tile([G, 1], f32, tag="grstd", name="grstd")
    nc.scalar.activation(out=grstd, in_=gvar, func=AF.Sqrt, bias=eps32, scale=1.0)
    nc.vector.reciprocal(grstd, grstd)
    gsr = stats.tile([G, 2], f32, tag="gsr", name="gsr")  # [gmean, grstd]
    nc.vector.tensor_copy(gsr[:, 0:1], gs[:, 0:1])
    nc.vector.tensor_copy(gsr[:, 1:2], grstd)
    Asb, Bsb = [], []
    for ci in range(CT):
        # independent per-ci chains: split across DVE/Pool to shorten the
        # serial small-op tail between the stats barrier and the first conv
        eng = nc.vector if ci % 2 == 0 else nc.gpsimd
        mrps = ps_small.tile([P, 2], f32, tag="small", name=f"mrps{ci}")
        nc.tensor.matmul(mrps, lhsT=gmTsb[ci], rhs=gsr, start=True, stop=True)
        mr = stats.tile([P, 2], f32, tag=f"mr{ci}", name=f"mr{ci}")
        nc.vector.tensor_copy(mr, mrps)
        a = stats.tile([P, 1], f32, tag=f"A{ci}", name=f"A{ci}")
        eng.tensor_mul(a, mr[:, 1:2], nwsb[ci])
        bb = stats.tile([P, 1], f32, tag=f"Bf{ci}", name=f"Bf{ci}")
        eng.tensor_mul(bb, mr[:, 0:1], a)
        eng.tensor_sub(bb, nbsb[ci], bb)
        Asb.append(a)
        Bsb.append(bb)


    # ---- stage 2: normalize+pack H8, then K/V/Q convs (all fp8 DoubleRow) ----
    # h8 superchunk tiles [128, 2, 1024]; conv rhs slices are [128, 2, 512].
    # psum evictions round-robin over DVE/Act/Pool; V-conv psum borrows the
    # (stage-3-only) ps_att pool to relieve ps_work slot pressure.
    ev = {"i": 0}
    ev_engs = [nc.vector, nc.scalar]  # GPSIMD cannot access PSUM

    def evict(dst, src):
        e = ev_engs[ev["i"] % 2]
        ev["i"] += 1
        if e is nc.scalar:
            e.copy(out=dst, in_=src)
        else:
            e.tensor_copy(dst, src)

    for j2 in range(NSC):
        h8 = [
            h8p.tile([P, 2, 2 * NB], fp8, tag=f"h8{pt}", name=f"h8{pt}_{j2}")
            for pt in range(2)
        ]
        for ci in range(CT):
            pt, pl = divmod(ci, 2)
            eng = nc.vector if ci % 2 == 0 else nc.gpsimd
            eng.tensor_scalar(
                out=h8[pt][:, pl, :],
                in0=xt[(ci, j2)],
                scalar1=Asb[ci],
                scalar2=Bsb[ci],
                op0=OP.mult,
                op1=OP.add,
            )
        for jj in range(2):
            j = 2 * j2 + jj
            h8s = [h8[pt][:, :, jj * NB : (jj + 1) * NB] for pt in range(2)]
            # K conv: [c_out, tokens]
            for co in range(CT):
                pk = ps_work.tile([P, NB], f32, tag="work", name=f"pk{j}_{co}")
                nc.tensor.matmul(
                    pk, lhsT=w8["k"][0][:, :, co * P : (co + 1) * P], rhs=h8s[0],
                    start=True, stop=False, perf_mode=DR,
                )
                nc.tensor.matmul(
                    pk, lhsT=w8["k"][1][:, :, co * P : (co + 1) * P], rhs=h8s[1],
                    start=False, stop=True, perf_mode=DR,
                )
                pt, pl = divmod(co, 2)
                evict(K8[pt][:, pl, j * NB : (j + 1) * NB], pk)
            # V conv: [tokens, c_out]
            for sub in range(NB // P):
                sg = j * (NB // P) + sub
                t, pl = divmod(sg, 2)
                pv = ps_att.tile([P, NB], f32, tag="att", name=f"pv{j}_{sub}")
                nc.tensor.matmul(
                    pv, lhsT=h8s[0][:, :, sub * P : (sub + 1) * P], rhs=w8["v"][0],
                    start=True, stop=False, perf_mode=DR,
                )
                nc.tensor.matmul(
                    pv, lhsT=h8s[1][:, :, sub * P : (sub + 1) * P], rhs=w8["v"][1],
                    start=False, stop=True, perf_mode=DR,
                )
                evict(Vt8[t][:, pl, 0:C], pv)
            # Q conv (first NQ tokens only)
            if j < NQ // NB:
                for co in range(CT):
                    pq = ps_work.tile([P, NB], f32, tag="work", name=f"pq{j}_{co}")
                    nc.tensor.matmul(
                        pq, lhsT=w8["q"][0][:, :, co * P : (co + 1) * P], rhs=h8s[0],
                        start=True, stop=False, perf_mode=DR,
                    )
                    nc.tensor.matmul(
                        pq, lhsT=w8["q"][1][:, :, co * P : (co + 1) * P], rhs=h8s[1],
                        start=False, stop=True, perf_mode=DR,
                    )
                    pt, pl = divmod(co, 2)
                    dst = Q8[pt][:, pl, j * NB : (j + 1) * NB]
                    e = ev_engs[ev["i"] % 2]
                    ev["i"] += 1
                    if e is nc.scalar:
                        e.activation(
                            out=dst, in_=pq, func=AF.Identity,
                            bias=bqcol[co], scale=1.0,
                        )
                    else:
                        e.tensor_scalar_add(out=dst, in0=pq, scalar1=bqcol[co])

    # ---- stage 3: attention + proj per query block ----
    # Software pipeline: PV lags exp by 2 key-tile pairs so the PE never waits
    # on a fresh exp except at the very last pair, and the previous qb's
    # epilogue (1/rowsum -> normalize -> proj -> residual -> store) is spread
    # over the first ~8 S/exp slots of the current qb.
    def _epi_recip(qb, rsacc2):
        rs = ps_small.tile([1, QBW], f32, tag="small", name=f"rs{qb}")
        nc.tensor.matmul(rs, lhsT=ones_col, rhs=rsacc2[:, 0:QBW], start=True,
                         stop=False)
        nc.tensor.matmul(rs, lhsT=ones_col, rhs=rsacc2[:, QBW : 2 * QBW],
                         start=False, stop=True)
        rs_sb = wk.tile([1, QBW], f32r, tag="rssb", name=f"rssb{qb}", bufs=2)
        with nc.allow_low_precision(reason="f32r == f32 bits; PE bcast operand"):
            nc.vector.reciprocal(rs_sb, rs)
        return rs_sb

    def _epi_rbc(qb, rs_sb):
        rbc = ps_small.tile([P, QBW], f32, tag="small", name=f"rbc{qb}")
        if "rbc" in _skip:
            nc.vector.memset(rbc, 1.0)
        else:
            nc.tensor.matmul(rbc, lhsT=ones_row, rhs=rs_sb, start=True, stop=True)
        return rbc

    def _epi_muls(qb, att_ps, rbc):
        # DVE can read only one PSUM operand per op: land rbc in SBUF first
        rbc_sb = wk.tile([P, QBW], f32, tag="rbcsb", name=f"rbcsb{qb}", bufs=2)
        nc.vector.tensor_copy(rbc_sb, rbc)
        att8 = [
            wk.tile([P, 2, QBW], fp8, tag=f"att8{pt}", name=f"att8{qb}_{pt}", bufs=2)
            for pt in range(2)
        ]
        for co in range(CT):
            pt, pl = divmod(co, 2)
            nc.vector.tensor_mul(att8[pt][:, pl, :], att_ps[co], rbc_sb)
        return att8

    def _epi_proj1(qb, att8, co, fo):
        pp = ps_work.tile([P, QBW], f32, tag="work", name=f"pp{qb}_{co}")
        nc.tensor.matmul(
            pp, lhsT=w8["p"][0][:, :, co * P : (co + 1) * P], rhs=att8[0],
            start=True, stop=False, perf_mode=DR,
        )
        nc.tensor.matmul(
            pp, lhsT=w8["p"][1][:, :, co * P : (co + 1) * P], rhs=att8[1],
            start=False, stop=True, perf_mode=DR,
        )
        # fo = (pp + bpe) + x   (proj bias incl. host-folded Wp@bv; psum
        # input so DVE only - GPSIMD cannot access PSUM)
        nc.vector.scalar_tensor_tensor(
            out=fo[:, co, :], in0=pp, scalar=bpesb[co], in1=xt[(co, qb // 2)][
                :, (qb % 2) * QBW : (qb % 2 + 1) * QBW
            ],
            op0=OP.add, op1=OP.add,
        )
        if co == CT - 1:
            # one store per qb on the sync queue (a DMA issue stalls the
            # issuing queue ~1.26us; Act must keep streaming exps); the last
            # qb splits halves across both queues to shorten the tail
            dst = out_d[:, :, qb * QBW : (qb + 1) * QBW]
            if qb == NQB - 1:
                nc.sync.dma_start(out=dst[:, 0:2, :], in_=fo[:, 0:2, :])
                nc.scalar.dma_start(out=dst[:, 2:4, :], in_=fo[:, 2:4, :])
            else:
                nc.sync.dma_start(out=dst, in_=fo)

    def _pv(qb, att_ps, Vt8t, p8t, t):
        for co in range(CT):
            nc.tensor.matmul(
                att_ps[co], lhsT=Vt8t[:, :, co * P : (co + 1) * P], rhs=p8t,
                start=(t == 0), stop=(t == NPAIR - 1), perf_mode=DR,
            )

    prev = None  # (qb, att_ps, rs) awaiting epilogue
    for qb in range(NQB):
        q8s = [Q8[pt][:, :, qb * QBW : (qb + 1) * QBW] for pt in range(2)]
        att_ps = None
        rsacc2 = None
        p8t = None
        p8tiles = {}
        e_rssb = e_rbc = e_att8 = None
        e_fo = None
        for nt in range(NKT):
            t, pl = divmod(nt, 2)
            st = ps_work.tile([P, QBW], f32, tag="work", name=f"st{qb}_{nt}")
            nc.tensor.matmul(
                st, lhsT=K8[0][:, :, nt * P : (nt + 1) * P], rhs=q8s[0],
                start=True, stop=False, perf_mode=DR,
            )
            nc.tensor.matmul(
                st, lhsT=K8[1][:, :, nt * P : (nt + 1) * P], rhs=q8s[1],
                start=False, stop=True, perf_mode=DR,
            )
            if pl == 0:
                p8t = p8p.tile(
                    [P, 2, QBW], fp8, tag="p8", name=f"p8_{qb}_{t}", bufs=4
                )
                p8tiles[t] = p8t
            nc.scalar.activation(
                out=p8t[:, pl, :], in_=st, func=AF.Exp,
                bias=nshift, scale=ISQ,
            )
            if prev is not None:
                if nt == 0:
                    e_rssb = _epi_recip(prev[0], prev[2])
                    pass
                elif nt == 1:
                    e_rbc = _epi_rbc(prev[0], e_rssb)
                elif nt == 2:
                    e_att8 = _epi_muls(prev[0], prev[1], e_rbc)
                elif 4 <= nt <= 7:
                    if nt == 4:
                        e_fo = wk.tile(
                            [P, CT, QBW], f32, tag="fo", name=f"fo{prev[0]}", bufs=2
                        )
                    _epi_proj1(prev[0], e_att8, nt - 4, e_fo)
                    if nt == 7:
                        prev = None
            if nt == 3:
                att_ps = [
                    ps_att.tile([P, QBW], f32, tag="att", name=f"attps{qb}_{co}")
                    for co in range(CT)
                ]
            if pl == 1:
                # rowsum: flat [128, 1024] DVE accumulate over pair planes
                # (PE rowsum matmuls are LDWEIGHTS-bound on HW)
                if nt == 1:
                    rsacc2 = wk.tile(
                        [P, 2 * QBW], f32r, tag="rsacc", name=f"rsacc{qb}", bufs=2
                    )
                    with nc.allow_low_precision(reason="f32 bits; PE collapse"):
                        nc.vector.tensor_copy(rsacc2, p8t)
                else:
                    with nc.allow_low_precision(reason="f32 bits; PE collapse"):
                        nc.vector.tensor_add(rsacc2, rsacc2, p8t)
            if nt >= 3 and pl == 1:
                tl = (nt - 3) // 2  # lagged pair: 0 at nt3, .., 14 at nt31
                _pv(qb, att_ps, Vt8[tl], p8tiles.pop(tl), tl)
        _pv(qb, att_ps, Vt8[NPAIR - 1], p8tiles.pop(NPAIR - 1), NPAIR - 1)
        prev = (qb, att_ps, rsacc2)
    e_rssb = _epi_recip(prev[0], prev[2])
    e_rbc = _epi_rbc(prev[0], e_rssb)
    e_att8 = _epi_muls(prev[0], prev[1], e_rbc)
    e_fo = wk.tile([P, CT, QBW], f32, tag="fo", name=f"fo{prev[0]}", bufs=2)
    for co in range(CT):
        _epi_proj1(prev[0], e_att8, co, e_fo)


def _build_program(reps=1):
    bass, bacc, tile, mybir, _ = _imports()
    f32 = mybir.dt.float32
    fp8 = mybir.dt.float8e4

    nc = bacc.Bacc("TRN2", target_bir_lowering=False, debug=False, num_devices=8)

    io = {}
    io["x"] = nc.dram_tensor("x", [C, N], f32, kind="ExternalInput").ap()
    io["cbuf"] = nc.dram_tensor("cbuf", [P, CBW], f32, kind="ExternalInput").ap()
    io["wall"] = nc.dram_tensor(
        "wall", [P, 4, 2, 2, C], fp8, kind="ExternalInput"
    ).ap()
    io["out"] = nc.dram_tensor("out", [P, CT, NQ], f32, kind="ExternalOutput").ap()
    nc._io = io

    with tile.TileContext(nc) as tc, ExitStack() as ctx:
        pools = {}
        pools["consts"] = ctx.enter_context(tc.tile_pool(name="consts", bufs=1))
        pools["xres"] = ctx.enter_context(tc.tile_pool(name="xres", bufs=1))
        pools["h8"] = ctx.enter_context(tc.tile_pool(name="h8", bufs=3))
        pools["kv8"] = ctx.enter_context(tc.tile_pool(name="kv8", bufs=1))
        pools["p8"] = ctx.enter_context(tc.tile_pool(name="p8", bufs=4))
        pools["work"] = ctx.enter_context(tc.tile_pool(name="work", bufs=2))
        pools["stats"] = ctx.enter_context(tc.tile_pool(name="stats", bufs=1))
        pools["bstp"] = ctx.enter_context(tc.tile_pool(name="bstp", bufs=1))
        pools["ps_work"] = ctx.enter_context(
            tc.tile_pool(name="ps_work", bufs=3, space="PSUM")
        )
        pools["ps_att"] = ctx.enter_context(
            tc.tile_pool(name="ps_att", bufs=4, space="PSUM")
        )
        pools["ps_small"] = ctx.enter_context(
            tc.tile_pool(name="ps_small", bufs=1, space="PSUM")
        )
        nc._pools = pools

        if reps > 1:
            with tc.For_i(0, reps, 1):
                _build_body(nc, tc, ctx, bass, tile, mybir)
        else:
            _build_body(nc, tc, ctx, bass, tile, mybir)

    nc.compile()
    return nc


@functools.lru_cache(maxsize=2)
def _get_nc(reps=1):
    return _build_program(reps)


def _pack_w8(w, e4):
    """[O, C] conv weight -> [2, 128, 2, C] fp8 lhsT pack (plane-major pairs).

    (pt, p, j, o): input channel c = pt*256 + j*128 + p, output channel o.
    """
    wT = np.ascontiguousarray(np.asarray(w, np.float32).T)  # [c_in, c_out]
    return np.ascontiguousarray(
        wT.reshape(2, 2, P, C).transpose(0, 2, 1, 3)
    ).astype(e4)


def _host_inputs(x, norm_w, norm_b, q_w, q_b, k_w, k_b, v_w, v_b, proj_w, proj_b):
    """Build the 8 per-core input maps."""
    import ml_dtypes

    e4 = ml_dtypes.float8_e4m3
    x = np.asarray(x)
    B = x.shape[0]
    xf = np.ascontiguousarray(x.reshape(B, C, N)).astype(np.float32)
    # f32 const buffer: gm[0:128] | nw[128:132] | nb[132:136] | bpe[136:140]
    # | gmT[140:652] (on partitions 0:32) | bq[652:656]
    cbuf = np.zeros((P, CBW), np.float32)
    for ci in range(CT):
        for c in range(P):
            cbuf[c, 32 * ci + (ci * P + c) // GSZ] = 1.0 / GSZ
            cbuf[(ci * P + c) // GSZ, 140 + P * ci + c] = 1.0
    cbuf[:, 128:132] = np.asarray(norm_w, np.float32).reshape(CT, P).T
    cbuf[:, 132:136] = np.asarray(norm_b, np.float32).reshape(CT, P).T
    bpe = np.asarray(proj_b, np.float32) + np.asarray(
        proj_w, np.float32
    ) @ np.asarray(v_b, np.float32)
    cbuf[:, 136:140] = bpe.reshape(CT, P).T
    # bq columns: (pt, pl) -> channels pt*256 + pl*128 + p
    cbuf[:, 652:656] = np.asarray(q_b, np.float32).reshape(2, 2, P).reshape(4, P).T
    # fp8 weight wall [P, m, pt, pl, c_out], m order q,k,v,p
    wall = np.zeros((P, 4, 2, 2, C), np.float32)
    for mi, w in enumerate((q_w, k_w, v_w, proj_w)):
        wT = np.ascontiguousarray(np.asarray(w, np.float32).T)  # [c_in, c_out]
        wall[:, mi] = wT.reshape(2, 2, P, C).transpose(2, 0, 1, 3)
    wall8 = np.ascontiguousarray(wall).astype(e4)
    shared = {"cbuf": cbuf, "wall": wall8}
    in_maps = []
    for core in range(8):
        b, hf = core // 2, core % 2
        if hf == 0:
            xp = xf[b]
        else:
            xp = np.concatenate([xf[b, :, NQ:], xf[b, :, :NQ]], axis=1)
        in_maps.append({"x": np.ascontiguousarray(xp), **shared})
    return in_maps


def kernel(**inputs):
    _, _, _, _, run_bass_kernel_spmd = _imports()
    nc = _get_nc()
    in_maps = _host_inputs(**inputs)
    res = run_bass_kernel_spmd(nc, in_maps, core_ids=list(range(8)))
    x = inputs["x"]
    B = x.shape[0]
    out = np.empty((B, C, N), np.float32)
    for core in range(8):
        b, hf = core // 2, core % 2
        # device out is [P, CT, NQ]: channel c = co*128 + p
        arr = np.asarray(res.results[core]["out"])
        out[b, :, hf * NQ : (hf + 1) * NQ] = arr.transpose(1, 0, 2).reshape(C, NQ)
    return out.reshape(x.shape)
